# revision 1
# baseline (speedup 1.0000x reference)
"""AutoCorrelation (Autoformer-style) sparse attention kernel for 8 trn2 cores.

Math (exact refactoring of the reference):
  mean_corr[b,j] = <(sum_i queries[b,i]) @ wq @ wk.T, keys[b,j]> / (H*L)
  top7 delays d_k + softmax weights w_k over mean_corr
  out[b,l]      = sum_k w_k * (values[b] @ wv @ wo)[(l+d_k) % L]
                = (sum_k w_k * values[b,(l+d_k)%L]) @ (wv@wo)

Sharding: core c handles batch b=c//2, output half h=c%2 (rows [h*1024, h*1024+1024)).
Host does layout-only prep (slicing / transpose); all FLOPs run on device.
The two big matmuls (W2 = wv@wo and vmix@W2) run in f32r; their inputs are
produced as real f32r tiles (walrus requires rounded producers).

Hardware constraints honored: a DMA carries at most one sync wait, so every DMA
writes a fresh SBUF region; scratch aliases spent regions of resident packs.
"""

import numpy as np
from contextlib import ExitStack

import concourse.bass as bass
import concourse.bacc as bacc
import concourse.mybir as mybir
import concourse.tile as tile
from concourse import masks
from concourse.bass_utils import run_bass_kernel_spmd

B, L, D, H = 4, 2048, 512, 8
HALF = L // 2          # 1024 output rows per core
KTOP = 7               # max(1, int(log(2048))) = 7
EXT = L + HALF         # values extended along L for wrap-free dynamic slicing
P = 128
FT = D // P            # 4 feature tiles
NT = L // P            # 16 sequence tiles
NH = HALF // P         # 8 output row chunks
F32 = mybir.dt.float32
F32R = mybir.dt.float32r
U32 = mybir.dt.uint32
I32 = mybir.dt.int32
AF = mybir.ActivationFunctionType
ALU = mybir.AluOpType
ENG = mybir.EngineType

# engine split points (DVE vs gpsimd)
SC_DVE = 11            # keys tiles 0..10 scored on DVE, 11..15 on gpsimd
MIX_DVE = 768          # mix columns [0,640) on DVE, [640,1024) on gpsimd


def _build():
    nc = bacc.Bacc()
    q_d = nc.dram_tensor("q", [L, D], F32, kind="ExternalInput")
    k_d = nc.dram_tensor("k", [L, D], F32, kind="ExternalInput")
    vt_d = nc.dram_tensor("vt", [P, FT, L], F32, kind="ExternalInput")
    wq_d = nc.dram_tensor("wq", [D, D], F32, kind="ExternalInput")
    wkT_d = nc.dram_tensor("wkT", [D, D], F32, kind="ExternalInput")
    wvT_d = nc.dram_tensor("wvT", [D, D], F32, kind="ExternalInput")
    wo_d = nc.dram_tensor("wo", [D, D], F32, kind="ExternalInput")
    ident_d = nc.dram_tensor("ident", [P, P], F32, kind="ExternalInput")
    cstc_d = nc.dram_tensor("cstc", [P, 1], F32, kind="ExternalInput")
    cstr_d = nc.dram_tensor("cstr", [1, 2 * P], F32, kind="ExternalInput")
    out_d = nc.dram_tensor("out", [HALF, D], F32, kind="ExternalOutput")

    with tile.TileContext(nc) as tc, ExitStack() as ctx:
        big = ctx.enter_context(tc.tile_pool(name="big", bufs=1))
        sm = ctx.enter_context(tc.tile_pool(name="sm", bufs=1))
        scr = ctx.enter_context(tc.tile_pool(name="scr", bufs=1))
        psA = ctx.enter_context(
            tc.tile_pool(name="psA", bufs=2, space=bass.MemorySpace.PSUM)
        )
        psB = ctx.enter_context(
            tc.tile_pool(name="psB", bufs=2, space=bass.MemorySpace.PSUM)
        )

        qdr = q_d.rearrange("(t p) c -> p t c", p=P)
        kdr = k_d.rearrange("(t p) c -> p t c", p=P)

        # ---- resident input packs; DMAs in priority order, fresh targets -
        wktp = big.tile([P, FT, D], F32, tag="wktp")
        nc.sync.dma_start(wktp[:], wkT_d.rearrange("(m p) c -> p m c", p=P))
        wqp = big.tile([P, FT, D], F32, tag="wqp")
        nc.sync.dma_start(wqp[:], wq_d.rearrange("(m p) c -> p m c", p=P))

        qpack = big.tile([P, NT, D], F32, tag="qpack")
        for j in range(8):
            nc.sync.dma_start(
                qpack[:, 2 * j : 2 * j + 2, :], qdr[:, 2 * j : 2 * j + 2, :]
            )
        kpack = big.tile([P, NT, D], F32, tag="kpack")
        for j in range(8):
            nc.sync.dma_start(
                kpack[:, 2 * j : 2 * j + 2, :], kdr[:, 2 * j : 2 * j + 2, :]
            )

        # values (transposed); circular extension built on ACT
        vt_sb = big.tile([P, FT, EXT], F32, tag="vt")
        nc.sync.dma_start(vt_sb[:, 0:2, 0:L], vt_d[:, 0:2, :])
        nc.sync.dma_start(vt_sb[:, 2:4, 0:L], vt_d[:, 2:4, :])
        nc.scalar.copy(vt_sb[:, :, L:EXT], vt_sb[:, :, 0:HALF])

        wvp = big.tile([P, FT, D], F32, tag="wvp")
        nc.sync.dma_start(wvp[:], wvT_d.rearrange("(m p) c -> p m c", p=P))
        wop = big.tile([P, FT, D], F32, tag="wop")
        nc.sync.dma_start(wop[:], wo_d.rearrange("(m p) c -> p m c", p=P))

        # ---- small packed tiles -----------------------------------------
        aux = sm.tile([P, 352], F32, tag="aux")
        ident = aux[:, 0:P]
        s_tile = aux[:, P : P + NT]
        qsumT = aux[:, 144:148]
        t1T = aux[:, 148:152]
        wbc = aux[:, 152:159]
        ones_col = aux[:, 159:160]
        sT_sb = aux[0:16, 160:288]
        vals8 = aux[0:1, 288:296]
        ex = aux[0:1, 296:303]
        negm = aux[0:1, 303:304]
        se = aux[0:1, 304:305]
        rse = aux[0:1, 305:306]
        w_sb = aux[0:1, 306:313]
        idx8 = aux[0:1, 320:328].bitcast(U32)
        nc.sync.dma_start(ident, ident_d[:])
        nc.sync.dma_start(ones_col, cstc_d[:])
        vec = sm.tile([1, 768], F32, tag="vec")
        qsum_sb = vec[0:1, 0:D]
        ones_row = vec[0:1, 512:640]
        scl_row = vec[0:1, 640:768]
        nc.sync.dma_start(vec[0:1, 512:768], cstr_d[:])

        s_flat = sm.tile([1, L], F32, tag="s_flat")
        u_sb = s_flat[0:1, 0:D]

        # ---- qsum: tree-accumulate q tiles on DVE+gpsimd, then one matmul
        # DVE folds tiles 1..7 into region 0; gpsimd folds 9..15 into 8.
        for t in range(1, 8):
            nc.vector.tensor_tensor(
                qpack[:, 0, :], qpack[:, t, :], qpack[:, 0, :], ALU.add
            )
        for t in range(9, NT):
            nc.vector.tensor_tensor(
                qpack[:, 8, :], qpack[:, t, :], qpack[:, 8, :], ALU.add
            )
        nc.vector.tensor_tensor(
            qpack[:, 0, :], qpack[:, 8, :], qpack[:, 0, :], ALU.add
        )
        ps_qsum = psA.tile([1, D], F32, tag="psa")
        nc.tensor.matmul(ps_qsum[:], ones_col, qpack[:, 0, :], start=True, stop=True)
        nc.scalar.copy(qsum_sb, ps_qsum[:])

        # qsumT [128,4] via 4 tiny K=1 matmuls: out = qsum_chunk^T @ [1]
        for c in range(FT):
            ps_qT = psA.tile([P, 1], F32, tag="psa")
            nc.tensor.matmul(
                ps_qT[:],
                vec[0:1, c * P : (c + 1) * P],
                ones_row[0:1, 0:1],
                start=True,
                stop=True,
            )
            nc.scalar.copy(qsumT[:, c : c + 1], ps_qT[:])

        # ---- t1T = (qsum @ wq) transposed, as [128,4] -------------------
        for jc in range(FT):
            ps_t1 = psA.tile([P, 1], F32, tag="psa")
            for mc in range(FT):
                nc.tensor.matmul(
                    ps_t1[:],
                    wqp[:, mc, jc * P : (jc + 1) * P],
                    qsumT[:, mc : mc + 1],
                    start=(mc == 0),
                    stop=(mc == FT - 1),
                )
            nc.scalar.copy(t1T[:, jc : jc + 1], ps_t1[:])

        # ---- u[1,512] = t1 @ wk.T (fp32) --------------------------------
        ps_u = psA.tile([1, D], F32, tag="psa")
        for mc in range(FT):
            nc.tensor.matmul(
                ps_u[:],
                t1T[:, mc : mc + 1],
                wktp[:, mc, :],
                start=(mc == 0),
                stop=(mc == FT - 1),
            )
        nc.scalar.copy(u_sb, ps_u[:])

        # broadcast u/(H*L) along partitions -> [128,512]
        ps_ub = psA.tile([P, D], F32, tag="psa")
        nc.tensor.matmul(ps_ub[:], scl_row, u_sb, start=True, stop=True)
        ub_sb = sm.tile([P, D], F32, tag="ub")
        nc.scalar.copy(ub_sb[:], ps_ub[:])

        # ---- scores s[128,16]: s[p,t] = <keys[t*128+p], u>/(H*L) --------
        # dead outputs land in spent qpack regions
        for t in range(NT):
            nc.vector.tensor_tensor(
                qpack[:, 1, :], kpack[:, t, :], ub_sb[:], ALU.mult
            )
            nc.vector.tensor_reduce(
                s_tile[:, t : t + 1], qpack[:, 1, :], mybir.AxisListType.X, ALU.add
            )

        # ---- flatten scores to [1,2048]: l = t*128+p --------------------
        ps_sT = psA.tile([NT, P], F32, tag="psa")
        nc.tensor.transpose(ps_sT[:], s_tile, ident)
        nc.scalar.copy(sT_sb[:], ps_sT[:])
        nc.sync.dma_start(s_flat[0:1, :], sT_sb[:])

        # ---- top-8 values + indices (descending), softmax over first 7 --
        nc.vector.max(vals8, s_flat[:])
        nc.vector.max_index(idx8, vals8, s_flat[:])

        nc.vector.tensor_scalar_mul(negm, vals8[0:1, 0:1], -1.0)
        nc.scalar.activation(ex, vals8[0:1, 0:KTOP], AF.Exp, bias=negm)
        nc.vector.tensor_reduce(se, ex, mybir.AxisListType.X, ALU.add)
        nc.vector.reciprocal(rse, se)
        nc.vector.tensor_scalar_mul(w_sb, ex, rse)

        # broadcast weights along partitions -> [128,7]
        ps_wbc = psA.tile([P, KTOP], F32, tag="psa")
        nc.tensor.matmul(ps_wbc[:], ones_row, w_sb, start=True, stop=True)
        nc.scalar.copy(wbc, ps_wbc[:])

        # ---- delays into registers (one batched load per engine) -------
        _, dks = nc.values_load_multi_w_load_instructions(
            idx8[0:1, 0:KTOP].bitcast(I32),
            engines=(ENG.DVE, ENG.Activation),
            min_val=0,
            max_val=L - 1,
            skip_runtime_bounds_check=True,
        )

        # ---- weighted circular mix ---------------------------------------
        # cols [0,MIX_DVE): ACT k0 scaled copy, then DVE stt accumulation
        # cols [MIX_DVE,HALF): pool mul+add pairs in f32 scratch, DVE rounds
        MG = HALF - MIX_DVE
        va = big.tile([P, FT, MIX_DVE], F32, tag="va")
        vb = big.tile([P, FT, MG], F32, tag="vb")
        accg = qpack[:, 2:4, :].rearrange("p a b -> p (a b)").rearrange(
            "p (f l) -> p f l", f=FT
        )
        tmpg = qpack[:, 4:6, :].rearrange("p a b -> p (a b)").rearrange(
            "p (f l) -> p f l", f=FT
        )
        nc.scalar.mul(
            va[:], vt_sb[:, :, bass.ds(dks[0], MIX_DVE)], wbc[:, 0:1]
        )
        nc.scalar.mul(
            accg[:], vt_sb[:, :, bass.ds(dks[0] + MIX_DVE, MG)], wbc[:, 0:1]
        )
        for kk in range(1, KTOP):
            nc.vector.scalar_tensor_tensor(
                va[:],
                vt_sb[:, :, bass.ds(dks[kk], MIX_DVE)],
                wbc[:, kk : kk + 1],
                va[:],
                ALU.mult,
                ALU.add,
            )
            nc.vector.scalar_tensor_tensor(
                accg[:],
                vt_sb[:, :, bass.ds(dks[kk] + MIX_DVE, MG)],
                wbc[:, kk : kk + 1],
                accg[:],
                ALU.mult,
                ALU.add,
            )
        nc.vector.tensor_copy(vb[:], accg[:])

        # ---- W2 = wv @ wo (f32r, chunkwise real-tile casts) -------------
        w2 = big.tile([P, FT, D], F32, tag="w2")
        ps_w2 = [
            psB.tile([P, D], F32, tag="psb", bufs=4, name=f"ps_w2_{i}")
            for i in range(FT)
        ]
        for mc in range(FT):
            for ic in range(FT):
                nc.tensor.matmul(
                    ps_w2[ic][:],
                    wvp[:, mc, ic * P : (ic + 1) * P],
                    wop[:, mc, :],
                    start=(mc == 0),
                    stop=(mc == FT - 1),
                )
        for ic in range(FT):
            nc.scalar.copy(w2[:, ic, :], ps_w2[ic][:])

        # ---- out rows: out[l,:] = sum_f vmixT[f,l] * W2[f,:] ------------
        # out staging aliases the spent second half of kpack
        for lc in range(NH):
            ps_out = psB.tile([P, D], F32, tag="psb", bufs=4)
            for ft in range(FT):
                src = (
                    va[:, ft, lc * P : (lc + 1) * P]
                    if (lc + 1) * P <= MIX_DVE
                    else vb[:, ft, lc * P - MIX_DVE : (lc + 1) * P - MIX_DVE]
                )
                nc.tensor.matmul(
                    ps_out[:],
                    src,
                    w2[:, ft, :],
                    start=(ft == 0),
                    stop=(ft == FT - 1),
                )
            ot = kpack[:, 8 + lc, :]
            nc.scalar.copy(ot, ps_out[:])
            nc.sync.dma_start(out_d[lc * P : (lc + 1) * P, :], ot)

    return nc


_IDENT = np.eye(P, dtype=np.float32)
_CSTC = np.ones((P, 1), np.float32)
_CSTR = np.concatenate(
    [np.ones((1, P), np.float32), np.full((1, P), 1.0 / (H * L), np.float32)], axis=1
)
_NC = None
TRACE = False
_LAST_RESULTS = None


def _get_nc():
    global _NC
    if _NC is None:
        _NC = _build()
        _NC.finalize()
    return _NC


def kernel(queries, keys, values, wq, wk, wv, wo):
    nc = _get_nc()
    wkT = np.ascontiguousarray(wk.T)
    wvT = np.ascontiguousarray(wv.T)
    in_maps = []
    for c in range(8):
        b, h = divmod(c, 2)
        vrot = np.roll(values[b], -h * HALF, axis=0)
        vte = np.ascontiguousarray(vrot.T.reshape(FT, P, L).transpose(1, 0, 2))
        in_maps.append(
            {
                "q": np.ascontiguousarray(queries[b]),
                "k": np.ascontiguousarray(keys[b]),
                "vt": vte,
                "wq": np.ascontiguousarray(wq),
                "wkT": wkT,
                "wvT": wvT,
                "wo": np.ascontiguousarray(wo),
                "ident": _IDENT,
                "cstc": _CSTC,
                "cstr": _CSTR,
            }
        )
    global _LAST_RESULTS
    res = run_bass_kernel_spmd(nc, in_maps, list(range(8)), trace=TRACE)
    _LAST_RESULTS = res
    out = np.empty((B, L, D), np.float32)
    for c in range(8):
        b, h = divmod(c, 2)
        out[b, h * HALF : (h + 1) * HALF] = res.results[c]["out"]
    return out



# revision 18
# speedup vs baseline: 3.6415x; 3.6415x over previous
"""AutoCorrelation (Autoformer-style) sparse attention kernel for 8 trn2 cores.

Math (exact refactoring of the reference):
  mean_corr[b,j] = <qsum @ (wq @ wk.T), k[b,j]> / (H*L),  qsum = sum_i q[b,i]
  top7 delays d_k + softmax weights w_k over mean_corr
  out[b,l]      = (sum_k w_k * values[b,(l+d_k)%L]) @ (wv@wo)

Sharding: core c handles batch b=c//2, output half h=c%2 (rows [h*1024, ...)).
Host does layout/dtype-only prep (slice/transpose/fp16 cast); all FLOPs on
device.  All heavy matmuls run in fp16 (inputs quantized to fp16, fp32 psum
accumulation); verified numerically: top-7 delay sets match fp32 exactly on
the fixed inputs and overall rel err ~7e-4 (tolerance 2e-2).

Compute placement:
  PE : W1=wq@wkT, qsum, u=qsum@W1, scores=uT.kT, W2=wv@wo, part of the
       weighted circular mix (scaled-identity psum accumulation), out matmuls
  ACT: psum->sbuf copies/downcasts, softmax exp, mix scaled-copy stream
  DVE: topk (max/max_index), transп downcasts, vt wrap extension, weighted
       identities, mix stt chain + adds for ACT stream
  Pool: mix stt chain for its column range
"""

import numpy as np
from contextlib import ExitStack

import concourse.bass as bass
import concourse.bacc as bacc
import concourse.mybir as mybir
import concourse.tile as tile
from concourse.bass_utils import run_bass_kernel_spmd

B, L, D, H = 4, 2048, 512, 8
HALF = L // 2          # 1024 output rows per core
KTOP = 7               # max(1, int(log(2048))) = 7
EXT = L + HALF         # values extended along L for wrap-free dynamic slicing
P = 128
FT = D // P            # 4 feature tiles
NT = L // P            # 16 sequence tiles
F32 = mybir.dt.float32
F16 = mybir.dt.float16
U32 = mybir.dt.uint32
I32 = mybir.dt.int32
AF = mybir.ActivationFunctionType
ALU = mybir.AluOpType
ENG = mybir.EngineType

# ---- mix column-range split (cols of the 1024 output rows) --------------
# strategy: PE scaled-identity psum accumulation / ACT mul + DVE add pipe /
#           DVE stt chain / Pool stt chain.  Ranges on 128 boundaries.
MIX_PE = (0, 640)      # 5 out chunks (psum tiles split 512+128 per fc)
MIX_ACT = (640, 896)   # 2 out chunks
MIX_DVE = (896, 1024)  # 1 out chunk
MIX_POOL = (1024, 1024)  # Pool cannot run scalar_tensor_tensor (walrus)


def _build():
    nc = bacc.Bacc()
    qp_d = nc.dram_tensor("qp", [P, NT, D], F16, kind="ExternalInput")
    ktp_d = nc.dram_tensor("ktp", [P, FT, L], F16, kind="ExternalInput")
    vt_d = nc.dram_tensor("vt", [P, FT, L], F16, kind="ExternalInput")
    wqk_d = nc.dram_tensor("wqk", [P, 2 * FT, D], F16, kind="ExternalInput")
    wvo_d = nc.dram_tensor("wvo", [P, 2 * FT, D], F16, kind="ExternalInput")
    cst16_d = nc.dram_tensor("cst16", [P, P + 2], F16, kind="ExternalInput")
    cstr_d = nc.dram_tensor("cstr", [1, P + 8], F32, kind="ExternalInput")
    out_d = nc.dram_tensor("out", [HALF, D], F32, kind="ExternalOutput")

    with tile.TileContext(nc) as tc, ExitStack() as ctx:
        big = ctx.enter_context(tc.tile_pool(name="big", bufs=1))
        sm = ctx.enter_context(tc.tile_pool(name="sm", bufs=1))
        psA = ctx.enter_context(
            tc.tile_pool(name="psA", bufs=4, space=bass.MemorySpace.PSUM)
        )
        psB = ctx.enter_context(
            tc.tile_pool(name="psB", bufs=4, space=bass.MemorySpace.PSUM)
        )

        # ---- resident input packs; DMAs in priority order ---------------
        wqk = big.tile([P, 2 * FT, D], F16, tag="wqk")
        nc.sync.dma_start(wqk[:], wqk_d[:])
        wqtp = wqk[:, 0:FT, :]
        wktp = wqk[:, FT:2 * FT, :]

        cst16 = sm.tile([P, P + 2], F16, tag="cst16")
        nc.sync.dma_start(cst16[:], cst16_d[:])
        ident16 = cst16[:, 0:P]
        ones16 = cst16[:, P:P + 1]
        cstr = sm.tile([1, P + 8], F32, tag="cstr")
        nc.sync.dma_start(cstr[:], cstr_d[:])
        ones_row = cstr[0:1, 0:P]
        one1 = cstr[0:1, P:P + 1]

        qp = big.tile([P, NT, D], F16, tag="qp")
        nc.sync.dma_start(qp[:, 0:8, :], qp_d[:, 0:8, :])
        nc.sync.dma_start(qp[:, 8:NT, :], qp_d[:, 8:NT, :])

        ktp = big.tile([P, FT, L], F16, tag="ktp")
        nc.sync.dma_start(ktp[:, 0:2, :], ktp_d[:, 0:2, :])
        nc.sync.dma_start(ktp[:, 2:FT, :], ktp_d[:, 2:FT, :])

        wvo = big.tile([P, 2 * FT, D], F16, tag="wvo")
        nc.sync.dma_start(wvo[:], wvo_d[:])
        wvtp = wvo[:, 0:FT, :]
        wop = wvo[:, FT:2 * FT, :]

        vtE = big.tile([P, FT, EXT], F16, tag="vtE")
        nc.sync.dma_start(vtE[:, :, 0:HALF], vt_d[:, :, 0:HALF])
        nc.sync.dma_start(vtE[:, :, HALF:L], vt_d[:, :, HALF:L])

        # ---- small sbuf tiles -------------------------------------------
        w1_16 = big.tile([P, FT, D], F16, tag="w1")
        w2_16 = big.tile([P, FT, D], F16, tag="w2")
        aux = sm.tile([P, 8], F32, tag="aux")
        wbc = aux[:, 0:7]                # broadcast weights [128,7]
        qsumT16 = sm.tile([P, 8], F16, tag="qsT")   # [:,0:4] qsumT, [:,4:8] uT
        uT16 = qsumT16[:, 4:8]
        srow = sm.tile([1, L + 64 + 2 * D], F32, tag="srow")
        qsum_sb = srow[0:1, L + 64:L + 64 + D]
        u_sb = srow[0:1, L + 64 + D:L + 64 + 2 * D]
        s_flat = srow[0:1, 0:L]
        vals8 = srow[0:1, L:L + 8]
        ex = srow[0:1, L + 8:L + 15]
        negm = srow[0:1, L + 16:L + 17]
        se = srow[0:1, L + 17:L + 18]
        rse = srow[0:1, L + 18:L + 19]
        w_sb = srow[0:1, L + 19:L + 26]
        idx8 = srow[0:1, L + 32:L + 40].bitcast(U32)
        wI16 = sm.tile([P, KTOP * P], F16, tag="wI")
        acc16 = big.tile([P, FT, HALF], F16, tag="acc16")
        tk16 = big.tile([P, 2, FT, MIX_ACT[1] - MIX_ACT[0]], F16, tag="tk16")
        vmx16 = big.tile([P, FT, MIX_PE[1] - MIX_PE[0]], F16, tag="vmx16")

        # ---- W1 = wq @ wk.T (fp16), scaled by 1/(H*L) at downcast -------
        ps_w1 = [psA.tile([P, D], F32, tag="psa", bufs=4, name=f"ps_w1_{i}") for i in range(FT)]
        for mc in range(FT):
            for ic in range(FT):
                nc.tensor.matmul(
                    ps_w1[ic][:],
                    wqtp[:, mc, ic * P:(ic + 1) * P],
                    wktp[:, mc, :],
                    start=(mc == 0),
                    stop=(mc == FT - 1),
                )
        # keep W1 at natural scale: scaling by 1/(H*L) here would push the
        # fp16 entries into subnormals (catastrophic rounding, flips topk);
        # the 1/(H*L) moves into the softmax scale/bias instead
        for ic in range(FT):
            nc.scalar.copy(w1_16[:, ic, :], ps_w1[ic][:])

        # ---- qsum = ones^T @ q  (psum f32) ------------------------------
        ps_qsum = psA.tile([1, D], F32, tag="psa", bufs=4, name="ps_qsum")
        for t in range(NT):
            nc.tensor.matmul(
                ps_qsum[:], ones16, qp[:, t, :],
                start=(t == 0), stop=(t == NT - 1),
            )
        nc.scalar.copy(qsum_sb, ps_qsum[:])

        # qsumT16 [128,4] via 4 tiny K=1 matmuls + DVE downcasts
        ps_qT = [psA.tile([P, 1], F32, tag="psa", bufs=4, name=f"ps_qT{c}") for c in range(FT)]
        for c in range(FT):
            nc.tensor.matmul(
                ps_qT[c][:], qsum_sb[0:1, c * P:(c + 1) * P], one1,
                start=True, stop=True,
            )
        for c in range(FT):
            nc.vector.tensor_copy(qsumT16[:, c:c + 1], ps_qT[c][:])

        # ---- uT directly: uT[j] = sum_c qsum[c] * W1[c,j] ---------------
        # (avoids the u row + transpose ping-pong: 16 tiny K-contraction
        # matmuls accumulate uT chunks straight into psum)
        ps_uT = [psA.tile([P, 1], F32, tag="psa", bufs=4, name=f"ps_uT{c}") for c in range(FT)]
        for cc in range(FT):
            for jc in range(FT):
                nc.tensor.matmul(
                    ps_uT[jc][:],
                    w1_16[:, cc, jc * P:(jc + 1) * P],
                    qsumT16[:, cc:cc + 1],
                    start=(cc == 0),
                    stop=(cc == FT - 1),
                )
        for c in range(FT):
            nc.vector.tensor_copy(uT16[:, c:c + 1], ps_uT[c][:])

        # ---- scores s[1,2048] = u . k_j  (4 psum banks of 512) ----------
        ps_s = [psB.tile([1, 512], F32, tag="psb", bufs=4, name=f"ps_s{j}") for j in range(FT)]
        for cc in range(FT):
            for j in range(FT):
                nc.tensor.matmul(
                    ps_s[j][:],
                    uT16[:, cc:cc + 1],
                    ktp[:, cc, j * 512:(j + 1) * 512],
                    start=(cc == 0),
                    stop=(cc == FT - 1),
                )

        # circular extension on Pool (idle until the mix starts)
        nc.gpsimd.tensor_copy(vtE[:, :, L:EXT], vtE[:, :, 0:HALF])

        # ---- W2 = wv @ wo (fp16) on PE while DVE runs the topk ----------
        # psA ring (scores own psB); downcasts on ACT
        ps_w2 = [psA.tile([P, D], F32, tag="psa", bufs=4, name=f"ps_w2_{i}") for i in range(FT)]
        for mc in range(FT):
            for ic in range(FT):
                nc.tensor.matmul(
                    ps_w2[ic][:],
                    wvtp[:, mc, ic * P:(ic + 1) * P],
                    wop[:, mc, :],
                    start=(mc == 0),
                    stop=(mc == FT - 1),
                )
        for ic in range(FT):
            nc.scalar.copy(w2_16[:, ic, :], ps_w2[ic][:])

        # post-W2 warmers: bridge the PE gap across the topk so the p-state
        # stays at full clock; ps_out0 is reset by its start=True acc later
        ps_out0 = psA.tile([P, D], F32, tag="psa", bufs=4, name="ps_out0")
        for i in range(16):
            nc.tensor.matmul(ps_out0[:, 0:P], ident16, ident16,
                             start=True, stop=True)

        for j in range(FT):
            dst = s_flat[0:1, j * 512:(j + 1) * 512]
            if j % 2 == 0:
                nc.scalar.copy(dst, ps_s[j][:])
            else:
                nc.vector.tensor_copy(dst, ps_s[j][:])

        # ---- top-8 + softmax over first 7 -------------------------------
        nc.vector.max(vals8, s_flat)
        nc.vector.max_index(idx8, vals8, s_flat)
        nc.vector.tensor_scalar_mul(negm, vals8[0:1, 0:1], -1.0 / (H * L))
        nc.scalar.activation(
            ex, vals8[0:1, 0:KTOP], AF.Exp, bias=negm, scale=1.0 / (H * L)
        )
        nc.vector.tensor_reduce(se, ex, mybir.AxisListType.X, ALU.add)
        nc.vector.reciprocal(rse, se)
        nc.vector.tensor_scalar_mul(w_sb, ex, rse)

        # broadcast weights along partitions -> wbc [128,7] f32
        ps_wbc = psB.tile([P, KTOP], F32, tag="psb", bufs=4, name="ps_wbc")
        nc.tensor.matmul(ps_wbc[:], ones_row, w_sb, start=True, stop=True)
        nc.scalar.copy(wbc, ps_wbc[:])


        # ---- delays into engine registers (right after topk; engine
        # queues are clean here so the cross-engine load doesn't stall) ----
        _, dks = nc.values_load_multi_w_load_instructions(
            idx8[0:1, 0:KTOP].bitcast(I32),
            engines=(ENG.PE, ENG.DVE, ENG.Activation),
            min_val=0,
            max_val=L - 1,
            skip_runtime_bounds_check=True,
        )

        # weighted identities for the PE mix stream (DVE, JIT per k)
        for kk in range(KTOP):
            nc.vector.tensor_scalar_mul(
                wI16[:, kk * P:(kk + 1) * P], ident16, wbc[:, kk:kk + 1]
            )



        # ---- mix: engine streams over column ranges ---------------------
        # PE: psum accumulation with weighted identities, 512-col groups
        a0, a1 = MIX_PE
        groups = []
        g = a0
        while g < a1:
            gw = min(512, a1 - g)
            groups.append((g, gw))
            g += gw
        for gi, (gb, gw) in enumerate(groups):
            for fc in range(FT):
                ps_mx = psB.tile([P, gw], F32, tag="psb", bufs=4,
                                 name=f"ps_mx{gi}_{fc}")
                for kk in range(KTOP):
                    nc.tensor.matmul(
                        ps_mx[:],
                        wI16[:, kk * P:(kk + 1) * P],
                        vtE[:, fc, bass.ds(dks[kk] + gb, gw)],
                        start=(kk == 0),
                        stop=(kk == KTOP - 1),
                    )
                # downcast as soon as this chunk's accumulation finishes;
                # alternate ACT/DVE so neither serializes the PE stream
                dst = vmx16[:, fc, gb - a0:gb - a0 + gw]
                if fc % 2 == 0:
                    nc.scalar.copy(dst, ps_mx[:])
                else:
                    nc.vector.tensor_copy(dst, ps_mx[:])

        # ACT mul stream + DVE add chain
        b0, b1 = MIX_ACT
        nb = b1 - b0
        accA = acc16[:, :, b0:b1]
        nc.scalar.mul(accA[:], vtE[:, :, bass.ds(dks[0] + b0, nb)], wbc[:, 0:1])
        for kk in range(1, KTOP):
            tkb = tk16[:, kk % 2]
            nc.scalar.mul(
                tkb[:], vtE[:, :, bass.ds(dks[kk] + b0, nb)], wbc[:, kk:kk + 1]
            )
            nc.vector.tensor_tensor(accA[:], tkb[:], accA[:], ALU.add)

        # DVE stt chain (disabled when the range is empty)
        c0, c1 = MIX_DVE
        if c1 > c0:
            ncd = c1 - c0
            accD = acc16[:, :, c0:c1]
            nc.vector.tensor_scalar_mul(
                accD[:], vtE[:, :, bass.ds(dks[0] + c0, ncd)], wbc[:, 0:1]
            )
            for kk in range(1, KTOP):
                nc.vector.scalar_tensor_tensor(
                    accD[:],
                    vtE[:, :, bass.ds(dks[kk] + c0, ncd)],
                    wbc[:, kk:kk + 1],
                    accD[:],
                    ALU.mult,
                    ALU.add,
                )


        # ---- out rows: out[l,:] = sum_f vmixT[f,l] * W2[f,:] ------------
        # psum -> sbuf staging (DMA cannot read PSUM); copies alternate
        # ACT/DVE; one DMA per pair of chunks.
        NH = HALF // P
        ostg = big.tile([P, NH, D], F32, tag="ostg")
        stage_eng = [0, 1, 0, 1, 0, 1, 0, 1]
        for lc in range(NH):
            lo = lc * P
            if lc == 0:
                ps_out = ps_out0
            else:
                ps_out = psA.tile([P, D], F32, tag="psa", bufs=4, name=f"ps_out{lc}")
            for ft in range(FT):
                if a0 <= lo < a1:
                    src = vmx16[:, ft, lo - a0:lo - a0 + P]
                else:
                    src = acc16[:, ft, lo:lo + P]
                nc.tensor.matmul(
                    ps_out[:], src, w2_16[:, ft, :],
                    start=(ft == 0), stop=(ft == FT - 1),
                )
            se_ = stage_eng[lc]
            if se_ == 0:
                nc.scalar.copy(ostg[:, lc, :], ps_out[:])
            elif se_ == 1:
                nc.vector.tensor_copy(ostg[:, lc, :], ps_out[:])
            else:
                nc.gpsimd.tensor_copy(ostg[:, lc, :], ps_out[:])
            nc.sync.dma_start(
                out_d.rearrange("(t p) c -> p t c", p=P)[:, lc:lc + 1, :],
                ostg[:, lc:lc + 1, :],
            )

    return nc


_NC = None
TRACE = False
_LAST_RESULTS = None


def _get_nc():
    global _NC
    if _NC is None:
        _NC = _build()
        _NC.finalize()
    return _NC


def _prep_consts():
    cst16 = np.zeros((P, P + 2), np.float16)
    cst16[:, 0:P] = np.eye(P, dtype=np.float16)
    cst16[:, P] = 1.0
    cstr = np.zeros((1, P + 8), np.float32)
    cstr[0, 0:P] = 1.0
    cstr[0, P] = 1.0
    return cst16, cstr


def kernel(queries, keys, values, wq, wk, wv, wo):
    nc = _get_nc()
    cst16, cstr = _prep_consts()
    f16 = np.float16

    def pack(m):
        # [512, 512] -> [128, 4, 512] with row index m = mc*128 + p
        return np.ascontiguousarray(
            m.reshape(FT, P, D).transpose(1, 0, 2).astype(f16)
        )

    wqk = np.ascontiguousarray(
        np.concatenate([pack(np.asarray(wq).T), pack(np.asarray(wk).T)], axis=1)
    )
    wvo = np.ascontiguousarray(
        np.concatenate([pack(np.asarray(wv).T), pack(np.asarray(wo))], axis=1)
    )

    in_maps = []
    for c in range(8):
        b, h = divmod(c, 2)
        qp = np.ascontiguousarray(
            queries[b].reshape(NT, P, D).transpose(1, 0, 2).astype(f16)
        )
        ktp = np.ascontiguousarray(
            keys[b].T.reshape(FT, P, L).transpose(1, 0, 2).astype(f16)
        )
        vrot = np.roll(values[b], -h * HALF, axis=0)
        vt = np.ascontiguousarray(
            vrot.T.reshape(FT, P, L).transpose(1, 0, 2).astype(f16)
        )
        in_maps.append(
            {
                "qp": qp,
                "ktp": ktp,
                "vt": vt,
                "wqk": wqk,
                "wvo": wvo,
                "cst16": cst16,
                "cstr": cstr,
            }
        )
    global _LAST_RESULTS
    res = run_bass_kernel_spmd(nc, in_maps, list(range(8)), trace=TRACE)
    _LAST_RESULTS = res
    out = np.empty((B, L, D), np.float32)
    for c in range(8):
        b, h = divmod(c, 2)
        out[b, h * HALF:(h + 1) * HALF] = res.results[c]["out"]
    return out


# revision 21
# speedup vs baseline: 3.6775x; 1.0099x over previous
"""AutoCorrelation (Autoformer-style) sparse attention kernel for 8 trn2 cores.

Math (exact refactoring of the reference):
  mean_corr[b,j] = <qsum @ (wq @ wk.T), k[b,j]> / (H*L),  qsum = sum_i q[b,i]
  top7 delays d_k + softmax weights w_k over mean_corr
  out[b,l]      = (sum_k w_k * values[b,(l+d_k)%L]) @ (wv@wo)

Sharding: core c handles batch b=c//2, output half h=c%2 (rows [h*1024, ...)).
Host does layout/dtype-only prep (slice/transpose/fp16 cast); all FLOPs on
device.  All heavy matmuls run in fp16 (inputs quantized to fp16, fp32 psum
accumulation); verified numerically: top-7 delay sets match fp32 exactly on
the fixed inputs and overall rel err ~7e-4 (tolerance 2e-2).

Compute placement:
  PE : W1=wq@wkT, qsum, u=qsum@W1, scores=uT.kT, W2=wv@wo, part of the
       weighted circular mix (scaled-identity psum accumulation), out matmuls
  ACT: psum->sbuf copies/downcasts, softmax exp, mix scaled-copy stream
  DVE: topk (max/max_index), transп downcasts, vt wrap extension, weighted
       identities, mix stt chain + adds for ACT stream
  Pool: mix stt chain for its column range
"""

import numpy as np
from contextlib import ExitStack

import concourse.bass as bass
import concourse.bacc as bacc
import concourse.mybir as mybir
import concourse.tile as tile
from concourse.bass_utils import run_bass_kernel_spmd

B, L, D, H = 4, 2048, 512, 8
HALF = L // 2          # 1024 output rows per core
KTOP = 7               # max(1, int(log(2048))) = 7
EXT = L + HALF         # values extended along L for wrap-free dynamic slicing
P = 128
FT = D // P            # 4 feature tiles
NT = L // P            # 16 sequence tiles
F32 = mybir.dt.float32
F16 = mybir.dt.float16
U32 = mybir.dt.uint32
I32 = mybir.dt.int32
AF = mybir.ActivationFunctionType
ALU = mybir.AluOpType
ENG = mybir.EngineType

# ---- mix column-range split (cols of the 1024 output rows) --------------
# strategy: PE scaled-identity psum accumulation / ACT mul + DVE add pipe /
#           DVE stt chain / Pool stt chain.  Ranges on 128 boundaries.
MIX_PE = (0, 640)      # 5 out chunks (psum tiles split 512+128 per fc)
MIX_ACT = (640, 896)   # 2 out chunks
MIX_DVE = (896, 1024)  # 1 out chunk
MIX_POOL = (1024, 1024)  # Pool cannot run scalar_tensor_tensor (walrus)


def _build():
    nc = bacc.Bacc()
    qp_d = nc.dram_tensor("qp", [P, NT, D], F16, kind="ExternalInput")
    ktp_d = nc.dram_tensor("ktp", [P, FT, L], F16, kind="ExternalInput")
    vt_d = nc.dram_tensor("vt", [P, FT, L], F16, kind="ExternalInput")
    wqk_d = nc.dram_tensor("wqk", [P, 2 * FT, D], F16, kind="ExternalInput")
    wvo_d = nc.dram_tensor("wvo", [P, 2 * FT, D], F16, kind="ExternalInput")
    cst16_d = nc.dram_tensor("cst16", [P, P + 2], F16, kind="ExternalInput")
    cstr_d = nc.dram_tensor("cstr", [1, P + 8], F32, kind="ExternalInput")
    out_d = nc.dram_tensor("out", [HALF, D], F32, kind="ExternalOutput")

    with tile.TileContext(nc) as tc, ExitStack() as ctx:
        big = ctx.enter_context(tc.tile_pool(name="big", bufs=1))
        sm = ctx.enter_context(tc.tile_pool(name="sm", bufs=1))
        psA = ctx.enter_context(
            tc.tile_pool(name="psA", bufs=4, space=bass.MemorySpace.PSUM)
        )
        psB = ctx.enter_context(
            tc.tile_pool(name="psB", bufs=4, space=bass.MemorySpace.PSUM)
        )

        # ---- resident input packs; DMAs in priority order ---------------
        wqk = big.tile([P, 2 * FT, D], F16, tag="wqk")
        nc.sync.dma_start(wqk[:], wqk_d[:])
        wqtp = wqk[:, 0:FT, :]
        wktp = wqk[:, FT:2 * FT, :]

        cst16 = sm.tile([P, P + 2], F16, tag="cst16")
        nc.sync.dma_start(cst16[:], cst16_d[:])
        ident16 = cst16[:, 0:P]
        ones16 = cst16[:, P:P + 1]
        cstr = sm.tile([1, P + 8], F32, tag="cstr")
        nc.sync.dma_start(cstr[:], cstr_d[:])
        ones_row = cstr[0:1, 0:P]
        one1 = cstr[0:1, P:P + 1]

        qp = big.tile([P, NT, D], F16, tag="qp")
        nc.sync.dma_start(qp[:, 0:8, :], qp_d[:, 0:8, :])
        nc.sync.dma_start(qp[:, 8:NT, :], qp_d[:, 8:NT, :])

        ktp = big.tile([P, FT, L], F16, tag="ktp")
        nc.sync.dma_start(ktp[:, 0:2, :], ktp_d[:, 0:2, :])
        nc.sync.dma_start(ktp[:, 2:FT, :], ktp_d[:, 2:FT, :])

        wvo = big.tile([P, 2 * FT, D], F16, tag="wvo")
        nc.sync.dma_start(wvo[:], wvo_d[:])
        wvtp = wvo[:, 0:FT, :]
        wop = wvo[:, FT:2 * FT, :]

        vtE = big.tile([P, FT, EXT], F16, tag="vtE")
        nc.sync.dma_start(vtE[:, :, 0:HALF], vt_d[:, :, 0:HALF])
        nc.sync.dma_start(vtE[:, :, HALF:L], vt_d[:, :, HALF:L])

        # ---- small sbuf tiles -------------------------------------------
        w1_16 = big.tile([P, FT, D], F16, tag="w1")
        w2_16 = big.tile([P, FT, D], F16, tag="w2")
        aux = sm.tile([P, 8], F32, tag="aux")
        wbc = aux[:, 0:7]                # broadcast weights [128,7]
        qsumT16 = sm.tile([P, 8], F16, tag="qsT")   # [:,0:4] qsumT, [:,4:8] uT
        uT16 = qsumT16[:, 4:8]
        srow = sm.tile([1, L + 64 + 2 * D], F32, tag="srow")
        qsum_sb = srow[0:1, L + 64:L + 64 + D]
        u_sb = srow[0:1, L + 64 + D:L + 64 + 2 * D]
        s_flat = srow[0:1, 0:L]
        vals8 = srow[0:1, L:L + 8]
        ex = srow[0:1, L + 8:L + 15]
        negm = srow[0:1, L + 16:L + 17]
        se = srow[0:1, L + 17:L + 18]
        rse = srow[0:1, L + 18:L + 19]
        w_sb = srow[0:1, L + 19:L + 26]
        idx8 = srow[0:1, L + 32:L + 40].bitcast(U32)
        wI16 = sm.tile([P, KTOP * P], F16, tag="wI")
        acc16 = big.tile([P, FT, HALF], F16, tag="acc16")
        tk16 = big.tile([P, 2, FT, MIX_ACT[1] - MIX_ACT[0]], F16, tag="tk16")
        vmx16 = big.tile([P, FT, MIX_PE[1] - MIX_PE[0]], F16, tag="vmx16")

        # ---- W1 = wq @ wk.T (fp16), scaled by 1/(H*L) at downcast -------
        ps_w1 = [psA.tile([P, D], F32, tag="psa", bufs=4, name=f"ps_w1_{i}") for i in range(FT)]
        for mc in range(FT):
            for ic in range(FT):
                nc.tensor.matmul(
                    ps_w1[ic][:],
                    wqtp[:, mc, ic * P:(ic + 1) * P],
                    wktp[:, mc, :],
                    start=(mc == 0),
                    stop=(mc == FT - 1),
                )
        # keep W1 at natural scale: scaling by 1/(H*L) here would push the
        # fp16 entries into subnormals (catastrophic rounding, flips topk);
        # the 1/(H*L) moves into the softmax scale/bias instead
        for ic in range(FT):
            nc.scalar.copy(w1_16[:, ic, :], ps_w1[ic][:])

        # ---- qsum = ones^T @ q  (psum f32) ------------------------------
        ps_qsum = psA.tile([1, D], F32, tag="psa", bufs=4, name="ps_qsum")
        for t in range(NT):
            nc.tensor.matmul(
                ps_qsum[:], ones16, qp[:, t, :],
                start=(t == 0), stop=(t == NT - 1),
            )
        nc.scalar.copy(qsum_sb, ps_qsum[:])

        # qsumT16 [128,4] via 4 tiny K=1 matmuls + DVE downcasts
        ps_qT = [psA.tile([P, 1], F32, tag="psa", bufs=4, name=f"ps_qT{c}") for c in range(FT)]
        for c in range(FT):
            nc.tensor.matmul(
                ps_qT[c][:], qsum_sb[0:1, c * P:(c + 1) * P], one1,
                start=True, stop=True,
            )
        for c in range(FT):
            nc.vector.tensor_copy(qsumT16[:, c:c + 1], ps_qT[c][:])

        # ---- uT directly: uT[j] = sum_c qsum[c] * W1[c,j] ---------------
        # (avoids the u row + transpose ping-pong: 16 tiny K-contraction
        # matmuls accumulate uT chunks straight into psum)
        ps_uT = [psA.tile([P, 1], F32, tag="psa", bufs=4, name=f"ps_uT{c}") for c in range(FT)]
        for cc in range(FT):
            for jc in range(FT):
                nc.tensor.matmul(
                    ps_uT[jc][:],
                    w1_16[:, cc, jc * P:(jc + 1) * P],
                    qsumT16[:, cc:cc + 1],
                    start=(cc == 0),
                    stop=(cc == FT - 1),
                )
        for c in range(FT):
            nc.vector.tensor_copy(uT16[:, c:c + 1], ps_uT[c][:])

        # ---- scores s[1,2048] = u . k_j  (4 psum banks of 512) ----------
        ps_s = [psB.tile([1, 512], F32, tag="psb", bufs=4, name=f"ps_s{j}") for j in range(FT)]
        # bank-major: bank j finishes after its 4 cc accs, so copies and the
        # max halves pipeline behind the still-running later banks
        for j in range(FT):
            for cc in range(FT):
                nc.tensor.matmul(
                    ps_s[j][:],
                    uT16[:, cc:cc + 1],
                    ktp[:, cc, j * 512:(j + 1) * 512],
                    start=(cc == 0),
                    stop=(cc == FT - 1),
                )

        # circular extension on Pool (idle until the mix starts)
        nc.gpsimd.tensor_copy(vtE[:, :, L:EXT], vtE[:, :, 0:HALF])

        # ---- W2 = wv @ wo (fp16) on PE while DVE runs the topk ----------
        # psA ring (scores own psB); downcasts on ACT
        ps_w2 = [psA.tile([P, D], F32, tag="psa", bufs=4, name=f"ps_w2_{i}") for i in range(FT)]
        for mc in range(FT):
            for ic in range(FT):
                nc.tensor.matmul(
                    ps_w2[ic][:],
                    wvtp[:, mc, ic * P:(ic + 1) * P],
                    wop[:, mc, :],
                    start=(mc == 0),
                    stop=(mc == FT - 1),
                )
        for ic in range(FT):
            nc.scalar.copy(w2_16[:, ic, :], ps_w2[ic][:])

        # post-W2 warmers: bridge the PE gap across the topk so the p-state
        # stays at full clock; ps_out0 is reset by its start=True acc later
        ps_out0 = psA.tile([P, D], F32, tag="psa", bufs=4, name="ps_out0")
        for i in range(12):
            nc.tensor.matmul(ps_out0[:, 0:P], ident16, ident16,
                             start=True, stop=True)

        for j in range(FT):
            dst = s_flat[0:1, j * 512:(j + 1) * 512]
            if j == 1:
                nc.vector.tensor_copy(dst, ps_s[j][:])
            else:
                nc.scalar.copy(dst, ps_s[j][:])

        # ---- top-8 + softmax over first 7 -------------------------------
        # max in two pipelined halves (each starts as soon as its two score
        # banks are copied), merged by an 8+8 -> top8 pass
        v8a = srow[0:1, L + 40:L + 48]
        v8b = srow[0:1, L + 48:L + 56]
        nc.vector.max(v8a, s_flat[0:1, 0:1024])
        nc.vector.max(v8b, s_flat[0:1, 1024:2048])
        nc.vector.max(vals8, srow[0:1, L + 40:L + 56])
        nc.vector.max_index(idx8, vals8, s_flat)
        nc.vector.tensor_scalar_mul(negm, vals8[0:1, 0:1], -1.0 / (H * L))
        nc.scalar.activation(
            ex, vals8[0:1, 0:KTOP], AF.Exp, bias=negm, scale=1.0 / (H * L)
        )
        nc.vector.tensor_reduce(se, ex, mybir.AxisListType.X, ALU.add)
        nc.vector.reciprocal(rse, se)
        nc.vector.tensor_scalar_mul(w_sb, ex, rse)

        # broadcast weights along partitions -> wbc [128,7] f32
        ps_wbc = psB.tile([P, KTOP], F32, tag="psb", bufs=4, name="ps_wbc")
        nc.tensor.matmul(ps_wbc[:], ones_row, w_sb, start=True, stop=True)
        nc.scalar.copy(wbc, ps_wbc[:])


        # ---- delays into engine registers (right after topk; engine
        # queues are clean here so the cross-engine load doesn't stall) ----
        _, dks = nc.values_load_multi_w_load_instructions(
            idx8[0:1, 0:KTOP].bitcast(I32),
            engines=(ENG.PE, ENG.DVE, ENG.Activation),
            min_val=0,
            max_val=L - 1,
            skip_runtime_bounds_check=True,
        )

        # weighted identities for the PE mix stream (DVE, JIT per k)
        for kk in range(KTOP):
            nc.vector.tensor_scalar_mul(
                wI16[:, kk * P:(kk + 1) * P], ident16, wbc[:, kk:kk + 1]
            )



        # ---- mix: engine streams over column ranges ---------------------
        # PE: psum accumulation with weighted identities, 512-col groups
        a0, a1 = MIX_PE
        groups = []
        g = a0
        while g < a1:
            gw = min(512, a1 - g)
            groups.append((g, gw))
            g += gw
        for gi, (gb, gw) in enumerate(groups):
            for fc in range(FT):
                ps_mx = psB.tile([P, gw], F32, tag="psb", bufs=4,
                                 name=f"ps_mx{gi}_{fc}")
                for kk in range(KTOP):
                    nc.tensor.matmul(
                        ps_mx[:],
                        wI16[:, kk * P:(kk + 1) * P],
                        vtE[:, fc, bass.ds(dks[kk] + gb, gw)],
                        start=(kk == 0),
                        stop=(kk == KTOP - 1),
                    )
                # downcast as soon as this chunk's accumulation finishes;
                # alternate ACT/DVE so neither serializes the PE stream
                dst = vmx16[:, fc, gb - a0:gb - a0 + gw]
                if fc % 2 == 0:
                    nc.scalar.copy(dst, ps_mx[:])
                else:
                    nc.vector.tensor_copy(dst, ps_mx[:])

        # ACT mul stream + DVE add chain
        b0, b1 = MIX_ACT
        nb = b1 - b0
        accA = acc16[:, :, b0:b1]
        nc.scalar.mul(accA[:], vtE[:, :, bass.ds(dks[0] + b0, nb)], wbc[:, 0:1])
        for kk in range(1, KTOP):
            tkb = tk16[:, kk % 2]
            nc.scalar.mul(
                tkb[:], vtE[:, :, bass.ds(dks[kk] + b0, nb)], wbc[:, kk:kk + 1]
            )
            nc.vector.tensor_tensor(accA[:], tkb[:], accA[:], ALU.add)

        # DVE stt chain (disabled when the range is empty)
        c0, c1 = MIX_DVE
        if c1 > c0:
            ncd = c1 - c0
            accD = acc16[:, :, c0:c1]
            nc.vector.tensor_scalar_mul(
                accD[:], vtE[:, :, bass.ds(dks[0] + c0, ncd)], wbc[:, 0:1]
            )
            for kk in range(1, KTOP):
                nc.vector.scalar_tensor_tensor(
                    accD[:],
                    vtE[:, :, bass.ds(dks[kk] + c0, ncd)],
                    wbc[:, kk:kk + 1],
                    accD[:],
                    ALU.mult,
                    ALU.add,
                )


        # ---- out rows: out[l,:] = sum_f vmixT[f,l] * W2[f,:] ------------
        # psum -> sbuf staging (DMA cannot read PSUM); copies alternate
        # ACT/DVE; one DMA per pair of chunks.
        NH = HALF // P
        ostg = big.tile([P, NH, D], F32, tag="ostg")
        stage_eng = [0, 1, 0, 1, 0, 1, 0, 1]
        for lc in range(NH):
            lo = lc * P
            if lc == 0:
                ps_out = ps_out0
            else:
                ps_out = psA.tile([P, D], F32, tag="psa", bufs=4, name=f"ps_out{lc}")
            for ft in range(FT):
                if a0 <= lo < a1:
                    src = vmx16[:, ft, lo - a0:lo - a0 + P]
                else:
                    src = acc16[:, ft, lo:lo + P]
                nc.tensor.matmul(
                    ps_out[:], src, w2_16[:, ft, :],
                    start=(ft == 0), stop=(ft == FT - 1),
                )
            se_ = stage_eng[lc]
            if se_ == 0:
                nc.scalar.copy(ostg[:, lc, :], ps_out[:])
            elif se_ == 1:
                nc.vector.tensor_copy(ostg[:, lc, :], ps_out[:])
            else:
                nc.gpsimd.tensor_copy(ostg[:, lc, :], ps_out[:])
            nc.sync.dma_start(
                out_d.rearrange("(t p) c -> p t c", p=P)[:, lc:lc + 1, :],
                ostg[:, lc:lc + 1, :],
            )

    return nc


_NC = None
TRACE = False
_LAST_RESULTS = None


def _get_nc():
    global _NC
    if _NC is None:
        _NC = _build()
        _NC.finalize()
    return _NC


def _prep_consts():
    cst16 = np.zeros((P, P + 2), np.float16)
    cst16[:, 0:P] = np.eye(P, dtype=np.float16)
    cst16[:, P] = 1.0
    cstr = np.zeros((1, P + 8), np.float32)
    cstr[0, 0:P] = 1.0
    cstr[0, P] = 1.0
    return cst16, cstr


def kernel(queries, keys, values, wq, wk, wv, wo):
    nc = _get_nc()
    cst16, cstr = _prep_consts()
    f16 = np.float16

    def pack(m):
        # [512, 512] -> [128, 4, 512] with row index m = mc*128 + p
        return np.ascontiguousarray(
            m.reshape(FT, P, D).transpose(1, 0, 2).astype(f16)
        )

    wqk = np.ascontiguousarray(
        np.concatenate([pack(np.asarray(wq).T), pack(np.asarray(wk).T)], axis=1)
    )
    wvo = np.ascontiguousarray(
        np.concatenate([pack(np.asarray(wv).T), pack(np.asarray(wo))], axis=1)
    )

    in_maps = []
    for c in range(8):
        b, h = divmod(c, 2)
        qp = np.ascontiguousarray(
            queries[b].reshape(NT, P, D).transpose(1, 0, 2).astype(f16)
        )
        ktp = np.ascontiguousarray(
            keys[b].T.reshape(FT, P, L).transpose(1, 0, 2).astype(f16)
        )
        vrot = np.roll(values[b], -h * HALF, axis=0)
        vt = np.ascontiguousarray(
            vrot.T.reshape(FT, P, L).transpose(1, 0, 2).astype(f16)
        )
        in_maps.append(
            {
                "qp": qp,
                "ktp": ktp,
                "vt": vt,
                "wqk": wqk,
                "wvo": wvo,
                "cst16": cst16,
                "cstr": cstr,
            }
        )
    global _LAST_RESULTS
    res = run_bass_kernel_spmd(nc, in_maps, list(range(8)), trace=TRACE)
    _LAST_RESULTS = res
    out = np.empty((B, L, D), np.float32)
    for c in range(8):
        b, h = divmod(c, 2)
        out[b, h * HALF:(h + 1) * HALF] = res.results[c]["out"]
    return out


# revision 28
# speedup vs baseline: 3.6924x; 1.0040x over previous
"""AutoCorrelation (Autoformer-style) sparse attention kernel for 8 trn2 cores.

Math (exact refactoring of the reference):
  mean_corr[b,j] = <qsum @ (wq @ wk.T), k[b,j]> / (H*L),  qsum = sum_i q[b,i]
  top7 delays d_k + softmax weights w_k over mean_corr
  out[b,l]      = (sum_k w_k * values[b,(l+d_k)%L]) @ (wv@wo)

Sharding: core c handles batch b=c//2, output half h=c%2 (rows [h*1024, ...)).
Host does layout/dtype-only prep (slice/transpose/fp16 cast); all FLOPs on
device.  All heavy matmuls run in fp16 (inputs quantized to fp16, fp32 psum
accumulation); verified numerically: top-7 delay sets match fp32 exactly on
the fixed inputs and overall rel err ~7e-4 (tolerance 2e-2).

Compute placement:
  PE : W1=wq@wkT, qsum, u=qsum@W1, scores=uT.kT, W2=wv@wo, part of the
       weighted circular mix (scaled-identity psum accumulation), out matmuls
  ACT: psum->sbuf copies/downcasts, softmax exp, mix scaled-copy stream
  DVE: topk (max/max_index), transп downcasts, vt wrap extension, weighted
       identities, mix stt chain + adds for ACT stream
  Pool: mix stt chain for its column range
"""

import numpy as np
from contextlib import ExitStack

import concourse.bass as bass
import concourse.bacc as bacc
import concourse.mybir as mybir
import concourse.tile as tile
from concourse.bass_utils import run_bass_kernel_spmd

B, L, D, H = 4, 2048, 512, 8
HALF = L // 2          # 1024 output rows per core
KTOP = 7               # max(1, int(log(2048))) = 7
EXT = L + HALF         # values extended along L for wrap-free dynamic slicing
P = 128
FT = D // P            # 4 feature tiles
NT = L // P            # 16 sequence tiles
F32 = mybir.dt.float32
F16 = mybir.dt.float16
U32 = mybir.dt.uint32
I32 = mybir.dt.int32
AF = mybir.ActivationFunctionType
ALU = mybir.AluOpType
ENG = mybir.EngineType

# ---- mix column-range split (cols of the 1024 output rows) --------------
# strategy: PE scaled-identity psum accumulation / ACT mul + DVE add pipe /
#           DVE stt chain / Pool stt chain.  Ranges on 128 boundaries.
MIX_PE = (0, 640)      # 5 out chunks (psum tiles split 512+128 per fc)
MIX_ACT = (640, 896)   # 2 out chunks
MIX_DVE = (896, 1024)  # 1 out chunk
MIX_POOL = (1024, 1024)  # Pool cannot run scalar_tensor_tensor (walrus)


def _build():
    nc = bacc.Bacc()
    qp_d = nc.dram_tensor("qp", [P, NT, D], F16, kind="ExternalInput")
    ktp_d = nc.dram_tensor("ktp", [P, FT, L], F16, kind="ExternalInput")
    vt_d = nc.dram_tensor("vt", [P, FT, L], F16, kind="ExternalInput")
    wqk_d = nc.dram_tensor("wqk", [P, 2 * FT, D], F16, kind="ExternalInput")
    wvo_d = nc.dram_tensor("wvo", [P, 2 * FT, D], F16, kind="ExternalInput")
    cst16_d = nc.dram_tensor("cst16", [P, P + 2], F16, kind="ExternalInput")
    cstr_d = nc.dram_tensor("cstr", [1, P + 8], F32, kind="ExternalInput")
    out_d = nc.dram_tensor("out", [HALF, D], F32, kind="ExternalOutput")

    with tile.TileContext(nc) as tc, ExitStack() as ctx:
        big = ctx.enter_context(tc.tile_pool(name="big", bufs=1))
        sm = ctx.enter_context(tc.tile_pool(name="sm", bufs=1))
        psA = ctx.enter_context(
            tc.tile_pool(name="psA", bufs=4, space=bass.MemorySpace.PSUM)
        )
        psB = ctx.enter_context(
            tc.tile_pool(name="psB", bufs=4, space=bass.MemorySpace.PSUM)
        )

        # ---- resident input packs; DMAs in priority order ---------------
        wqk = big.tile([P, 2 * FT, D], F16, tag="wqk")
        nc.sync.dma_start(wqk[:], wqk_d[:])
        wqtp = wqk[:, 0:FT, :]
        wktp = wqk[:, FT:2 * FT, :]

        cst16 = sm.tile([P, P + 2], F16, tag="cst16")
        nc.sync.dma_start(cst16[:], cst16_d[:])
        ident16 = cst16[:, 0:P]
        ones16 = cst16[:, P:P + 1]
        cstr = sm.tile([1, P + 8], F32, tag="cstr")
        nc.sync.dma_start(cstr[:], cstr_d[:])
        ones_row = cstr[0:1, 0:P]
        one1 = cstr[0:1, P:P + 1]

        qp = big.tile([P, NT, D], F16, tag="qp")
        nc.sync.dma_start(qp[:, 0:8, :], qp_d[:, 0:8, :])
        nc.sync.dma_start(qp[:, 8:NT, :], qp_d[:, 8:NT, :])

        ktp = big.tile([P, FT, L], F16, tag="ktp")
        nc.sync.dma_start(ktp[:, 0:2, :], ktp_d[:, 0:2, :])
        nc.sync.dma_start(ktp[:, 2:FT, :], ktp_d[:, 2:FT, :])

        wvo = big.tile([P, 2 * FT, D], F16, tag="wvo")
        nc.sync.dma_start(wvo[:], wvo_d[:])
        wvtp = wvo[:, 0:FT, :]
        wop = wvo[:, FT:2 * FT, :]

        vtE = big.tile([P, FT, EXT], F16, tag="vtE")
        nc.sync.dma_start(vtE[:, :, 0:HALF], vt_d[:, :, 0:HALF])
        nc.sync.dma_start(vtE[:, :, HALF:L], vt_d[:, :, HALF:L])

        # ---- small sbuf tiles -------------------------------------------
        w1_16 = big.tile([P, FT, D], F16, tag="w1")
        w2_16 = big.tile([P, FT, D], F16, tag="w2")
        aux = sm.tile([P, 8], F32, tag="aux")
        wbc = aux[:, 0:7]                # broadcast weights [128,7]
        qsumT16 = sm.tile([P, 8], F16, tag="qsT")   # [:,0:4] qsumT, [:,4:8] uT
        uT16 = qsumT16[:, 4:8]
        srow = sm.tile([1, L + 64 + 2 * D], F32, tag="srow")
        qsum_sb = srow[0:1, L + 64:L + 64 + D]
        u_sb = srow[0:1, L + 64 + D:L + 64 + 2 * D]
        s_flat = srow[0:1, 0:L]
        vals8 = srow[0:1, L:L + 8]
        ex = srow[0:1, L + 8:L + 15]
        negm = srow[0:1, L + 16:L + 17]
        se = srow[0:1, L + 17:L + 18]
        rse = srow[0:1, L + 18:L + 19]
        w_sb = srow[0:1, L + 19:L + 26]
        idx8 = srow[0:1, L + 32:L + 40].bitcast(U32)
        wI16 = sm.tile([P, KTOP * P], F16, tag="wI")
        acc16 = big.tile([P, FT, HALF], F16, tag="acc16")
        tk16 = big.tile([P, 2, FT, MIX_ACT[1] - MIX_ACT[0]], F16, tag="tk16")
        vmx16 = big.tile([P, FT, MIX_PE[1] - MIX_PE[0]], F16, tag="vmx16")

        # ---- W1 = wq @ wk.T (fp16), scaled by 1/(H*L) at downcast -------
        ps_w1 = [psA.tile([P, D], F32, tag="psa", bufs=4, name=f"ps_w1_{i}") for i in range(FT)]
        for mc in range(FT):
            for ic in range(FT):
                nc.tensor.matmul(
                    ps_w1[ic][:],
                    wqtp[:, mc, ic * P:(ic + 1) * P],
                    wktp[:, mc, :],
                    start=(mc == 0),
                    stop=(mc == FT - 1),
                )
        # keep W1 at natural scale: scaling by 1/(H*L) here would push the
        # fp16 entries into subnormals (catastrophic rounding, flips topk);
        # the 1/(H*L) moves into the softmax scale/bias instead
        for ic in range(FT):
            nc.scalar.copy(w1_16[:, ic, :], ps_w1[ic][:])

        # ---- qsum = ones^T @ q  (psum f32) ------------------------------
        ps_qsum = psA.tile([1, D], F32, tag="psa", bufs=4, name="ps_qsum")
        for t in range(NT):
            nc.tensor.matmul(
                ps_qsum[:], ones16, qp[:, t, :],
                start=(t == 0), stop=(t == NT - 1),
            )
        nc.scalar.copy(qsum_sb, ps_qsum[:])

        # qsumT16 [128,4] via 4 tiny K=1 matmuls + DVE downcasts
        ps_qT = [psA.tile([P, 1], F32, tag="psa", bufs=4, name=f"ps_qT{c}") for c in range(FT)]
        for c in range(FT):
            nc.tensor.matmul(
                ps_qT[c][:], qsum_sb[0:1, c * P:(c + 1) * P], one1,
                start=True, stop=True,
            )
        for c in range(FT):
            nc.vector.tensor_copy(qsumT16[:, c:c + 1], ps_qT[c][:])

        # ---- uT directly: uT[j] = sum_c qsum[c] * W1[c,j] ---------------
        # (avoids the u row + transpose ping-pong: 16 tiny K-contraction
        # matmuls accumulate uT chunks straight into psum)
        ps_uT = [psA.tile([P, 1], F32, tag="psa", bufs=4, name=f"ps_uT{c}") for c in range(FT)]
        for cc in range(FT):
            for jc in range(FT):
                nc.tensor.matmul(
                    ps_uT[jc][:],
                    w1_16[:, cc, jc * P:(jc + 1) * P],
                    qsumT16[:, cc:cc + 1],
                    start=(cc == 0),
                    stop=(cc == FT - 1),
                )
        for c in range(FT):
            nc.vector.tensor_copy(uT16[:, c:c + 1], ps_uT[c][:])

        # ---- scores s[1,2048] = u . k_j  (4 psum banks of 512) ----------
        ps_s = [psB.tile([1, 512], F32, tag="psb", bufs=4, name=f"ps_s{j}") for j in range(FT)]
        # bank-major: bank j finishes after its 4 cc accs, so copies and the
        # max halves pipeline behind the still-running later banks
        for j in range(FT):
            for cc in range(FT):
                nc.tensor.matmul(
                    ps_s[j][:],
                    uT16[:, cc:cc + 1],
                    ktp[:, cc, j * 512:(j + 1) * 512],
                    start=(cc == 0),
                    stop=(cc == FT - 1),
                )

        # circular extension on Pool (idle until the mix starts)
        nc.gpsimd.tensor_copy(vtE[:, :, L:EXT], vtE[:, :, 0:HALF])

        # ---- W2 = wv @ wo (fp16) on PE while DVE runs the topk ----------
        # psA ring (scores own psB); downcasts on ACT
        ps_w2 = [psA.tile([P, D], F32, tag="psa", bufs=4, name=f"ps_w2_{i}") for i in range(FT)]
        for mc in range(FT):
            for ic in range(FT):
                nc.tensor.matmul(
                    ps_w2[ic][:],
                    wvtp[:, mc, ic * P:(ic + 1) * P],
                    wop[:, mc, :],
                    start=(mc == 0),
                    stop=(mc == FT - 1),
                )
        for ic in range(FT):
            nc.scalar.copy(w2_16[:, ic, :], ps_w2[ic][:])

        # post-W2 warmers: bridge the PE gap across the topk so the p-state
        # stays at full clock; ps_out0 is reset by its start=True acc later
        ps_out0 = psA.tile([P, D], F32, tag="psa", bufs=4, name="ps_out0")
        for i in range(12):
            nc.tensor.matmul(ps_out0[:, 0:P], ident16, ident16,
                             start=True, stop=True)

        for j in range(FT):
            dst = s_flat[0:1, j * 512:(j + 1) * 512]
            if j == 1:
                nc.vector.tensor_copy(dst, ps_s[j][:])
            else:
                nc.scalar.copy(dst, ps_s[j][:])

        # ---- top-8 + softmax over first 7 -------------------------------
        # max in two pipelined halves (each starts as soon as its two score
        # banks are copied), merged by an 8+8 -> top8 pass
        v8a = srow[0:1, L + 40:L + 48]
        v8b = srow[0:1, L + 48:L + 56]
        nc.vector.max(v8a, s_flat[0:1, 0:1024])
        nc.vector.max(v8b, s_flat[0:1, 1024:2048])
        nc.vector.max(vals8, srow[0:1, L + 40:L + 56])
        nc.vector.tensor_scalar_mul(negm, vals8[0:1, 0:1], -1.0 / (H * L))
        nc.vector.max_index(idx8, vals8, s_flat)
        # Exp computes its own sum via the ACT accumulator (one op less on
        # the DVE, which is busy with max_index)
        nc.scalar.activation(
            ex, vals8[0:1, 0:KTOP], AF.Exp, bias=negm, scale=1.0 / (H * L),
            accum_out=se,
        )
        nc.vector.reciprocal(rse, se)
        nc.vector.tensor_scalar_mul(w_sb, ex, rse)

        # broadcast weights along partitions -> wbc [128,7] f32
        ps_wbc = psB.tile([P, KTOP], F32, tag="psb", bufs=4, name="ps_wbc")
        nc.tensor.matmul(ps_wbc[:], ones_row, w_sb, start=True, stop=True)
        nc.scalar.copy(wbc, ps_wbc[:])


        # ---- delays into engine registers (right after topk; engine
        # queues are clean here so the cross-engine load doesn't stall) ----
        _, dks = nc.values_load_multi_w_load_instructions(
            idx8[0:1, 0:KTOP].bitcast(I32),
            engines=(ENG.PE, ENG.DVE, ENG.Activation),
            min_val=0,
            max_val=L - 1,
            skip_runtime_bounds_check=True,
        )

        # weighted identities for the PE mix stream (DVE, JIT per k)
        for kk in range(KTOP):
            nc.vector.tensor_scalar_mul(
                wI16[:, kk * P:(kk + 1) * P], ident16, wbc[:, kk:kk + 1]
            )



        # ---- mix: engine streams over column ranges ---------------------
        # PE: psum accumulation with weighted identities, 512-col groups
        a0, a1 = MIX_PE
        groups = []
        g = a0
        while g < a1:
            gw = min(512, a1 - g)
            groups.append((g, gw))
            g += gw
        for gi, (gb, gw) in enumerate(groups):
            for fc in range(FT):
                ps_mx = psB.tile([P, gw], F32, tag="psb", bufs=4,
                                 name=f"ps_mx{gi}_{fc}")
                for kk in range(KTOP):
                    nc.tensor.matmul(
                        ps_mx[:],
                        wI16[:, kk * P:(kk + 1) * P],
                        vtE[:, fc, bass.ds(dks[kk] + gb, gw)],
                        start=(kk == 0),
                        stop=(kk == KTOP - 1),
                    )
                # downcast as soon as this chunk's accumulation finishes;
                # alternate ACT/DVE so neither serializes the PE stream
                dst = vmx16[:, fc, gb - a0:gb - a0 + gw]
                if fc % 2 == 0:
                    nc.scalar.copy(dst, ps_mx[:])
                else:
                    nc.vector.tensor_copy(dst, ps_mx[:])

        # ACT mul stream + DVE add chain
        b0, b1 = MIX_ACT
        nb = b1 - b0
        accA = acc16[:, :, b0:b1]
        nc.scalar.mul(accA[:], vtE[:, :, bass.ds(dks[0] + b0, nb)], wbc[:, 0:1])
        for kk in range(1, KTOP):
            tkb = tk16[:, kk % 2]
            nc.scalar.mul(
                tkb[:], vtE[:, :, bass.ds(dks[kk] + b0, nb)], wbc[:, kk:kk + 1]
            )
            nc.vector.tensor_tensor(accA[:], tkb[:], accA[:], ALU.add)

        # DVE stt chain (disabled when the range is empty)
        c0, c1 = MIX_DVE
        if c1 > c0:
            ncd = c1 - c0
            accD = acc16[:, :, c0:c1]
            nc.vector.tensor_scalar_mul(
                accD[:], vtE[:, :, bass.ds(dks[0] + c0, ncd)], wbc[:, 0:1]
            )
            for kk in range(1, KTOP):
                nc.vector.scalar_tensor_tensor(
                    accD[:],
                    vtE[:, :, bass.ds(dks[kk] + c0, ncd)],
                    wbc[:, kk:kk + 1],
                    accD[:],
                    ALU.mult,
                    ALU.add,
                )


        # ---- out rows: out[l,:] = sum_f vmixT[f,l] * W2[f,:] ------------
        # psum -> sbuf staging (DMA cannot read PSUM); copies alternate
        # ACT/DVE; one DMA per pair of chunks.
        NH = HALF // P
        ostg = big.tile([P, NH, D], F32, tag="ostg")
        stage_eng = [0, 1, 0, 1, 0, 1, 0, 1]
        for lc in range(NH):
            lo = lc * P
            if lc == 0:
                ps_out = ps_out0
            else:
                ps_out = psA.tile([P, D], F32, tag="psa", bufs=4, name=f"ps_out{lc}")
            for ft in range(FT):
                if a0 <= lo < a1:
                    src = vmx16[:, ft, lo - a0:lo - a0 + P]
                else:
                    src = acc16[:, ft, lo:lo + P]
                nc.tensor.matmul(
                    ps_out[:], src, w2_16[:, ft, :],
                    start=(ft == 0), stop=(ft == FT - 1),
                )
            se_ = stage_eng[lc]
            if se_ == 0:
                nc.scalar.copy(ostg[:, lc, :], ps_out[:])
            elif se_ == 1:
                nc.vector.tensor_copy(ostg[:, lc, :], ps_out[:])
            else:
                nc.gpsimd.tensor_copy(ostg[:, lc, :], ps_out[:])
            nc.sync.dma_start(
                out_d.rearrange("(t p) c -> p t c", p=P)[:, lc:lc + 1, :],
                ostg[:, lc:lc + 1, :],
            )

    return nc


_NC = None
TRACE = False
_LAST_RESULTS = None


def _get_nc():
    global _NC
    if _NC is None:
        _NC = _build()
        _NC.finalize()
    return _NC


def _prep_consts():
    cst16 = np.zeros((P, P + 2), np.float16)
    cst16[:, 0:P] = np.eye(P, dtype=np.float16)
    cst16[:, P] = 1.0
    cstr = np.zeros((1, P + 8), np.float32)
    cstr[0, 0:P] = 1.0
    cstr[0, P] = 1.0
    return cst16, cstr


def kernel(queries, keys, values, wq, wk, wv, wo):
    nc = _get_nc()
    cst16, cstr = _prep_consts()
    f16 = np.float16

    def pack(m):
        # [512, 512] -> [128, 4, 512] with row index m = mc*128 + p
        return np.ascontiguousarray(
            m.reshape(FT, P, D).transpose(1, 0, 2).astype(f16)
        )

    wqk = np.ascontiguousarray(
        np.concatenate([pack(np.asarray(wq).T), pack(np.asarray(wk).T)], axis=1)
    )
    wvo = np.ascontiguousarray(
        np.concatenate([pack(np.asarray(wv).T), pack(np.asarray(wo))], axis=1)
    )

    in_maps = []
    for c in range(8):
        b, h = divmod(c, 2)
        qp = np.ascontiguousarray(
            queries[b].reshape(NT, P, D).transpose(1, 0, 2).astype(f16)
        )
        ktp = np.ascontiguousarray(
            keys[b].T.reshape(FT, P, L).transpose(1, 0, 2).astype(f16)
        )
        vrot = np.roll(values[b], -h * HALF, axis=0)
        vt = np.ascontiguousarray(
            vrot.T.reshape(FT, P, L).transpose(1, 0, 2).astype(f16)
        )
        in_maps.append(
            {
                "qp": qp,
                "ktp": ktp,
                "vt": vt,
                "wqk": wqk,
                "wvo": wvo,
                "cst16": cst16,
                "cstr": cstr,
            }
        )
    global _LAST_RESULTS
    res = run_bass_kernel_spmd(nc, in_maps, list(range(8)), trace=TRACE)
    _LAST_RESULTS = res
    out = np.empty((B, L, D), np.float32)
    for c in range(8):
        b, h = divmod(c, 2)
        out[b, h * HALF:(h + 1) * HALF] = res.results[c]["out"]
    return out
